# revision 43
# baseline (speedup 1.0000x reference)
"""DigitCaps kernel for 8 Trainium2 NeuronCores — PE-voting redesign.

Math (per batch b):
    U_hat[b,d,n,j] = sum_i W[d,n,j,i] * u[b,n,i]
    s[b,d,j]       = sum_n U_hat[b,d,n,j]
    A_sum[b,d,m]   = s[b,d,:] . U_hat[b,d,m,:] / sqrt(dp)
    C              = softmax_d(A_sum)
    S[b,d,j]       = sum_m (B_prior[d,m] + C[b,d,m]) * U_hat[b,d,m,j]
    out            = squash(S)

Sharding: data-parallel over batch, 2 batches per core; W/B_prior replicated
in bf16 (host-cast; rel err ~4e-3 << 2e-2 tolerance, halves the HBM read).

Votes run on the PE via a block-diagonal stationary operand: partitions hold
(n_sub=16, i=8) of a 16-n group; lhsT = u_blk[128, (b=2, n')=32] with
u_blk[(ns,i),(b,n')] = u[b,n(ns),i] * delta(ns==n'), rhs = W[128, (d,j)=160].
Out = U_hat[(b,n'), (d,j)] written to a 32-partition slice of PSUM; 4 groups
pack one bank to 128 partitions (a 64-n "chunk"; 18 chunks per core).

s is accumulated on the PE as bsel.T @ U2 (bsel[p,p'] = delta(b(p)==b(p')))
which lands s directly in row-broadcast layout [p=(b,m), (d,j)].

Phase 2 per chunk in layout [p=(g_sub,b,n'), (d,j)]: softmax over d is a
free-dim strided reduce (no cross-partition work); cbD = (exp*zr + Bp*bmask)
in one fused DVE op (Bp premasked on host); S via accumulating bf16 matmul
with lhsT = cbD[128, (b',d)=20]; diagonal d==d' extracted with a host mask.
"""

import math
import numpy as np

import concourse.bacc as bacc
import concourse.bass as bass
import concourse.tile as tile
from concourse import mybir
from concourse.bass_utils import run_bass_kernel_spmd

F32 = mybir.dt.float32
BF16 = mybir.dt.bfloat16
I32 = mybir.dt.int32
AX = mybir.AxisListType
OP = mybir.AluOpType
ACTF = mybir.ActivationFunctionType

B, N, DP = 16, 1152, 8
D, DD = 10, 16
NCORES = 8
BPC = B // NCORES            # 2 batches per core
FD = D * DD                  # 160 = (d,j)
NG = N // 16                 # 72 groups of 16 n's
NC = NG // 4                 # 18 chunks of 4 groups (64 n's)
NBD = BPC * D                # 20 = (b,d)
EPS = 1e-7
INV_SQRT_DP = 1.0 / math.sqrt(DP)

# W batches: KB chunks per DMA (fewer DMAs amortize the ~625ns serial HWDGE
# descriptor-gen per DMACopy; chunk-granular DMAs spent 13us in HWDGE)
KB = 3                       # chunks per W DMA batch
NB = NC // KB                # 6 batches
WCOLS = 4 * FD               # 640 W cols per chunk

# const tile free layout (bf16 cols):
# bsel 128 | diag 32 | dmask 160 | bias2 2 | u_c_all 144
C_BSEL, C_DIAG, C_DMASK, C_BIAS2, C_UC = 0, 128, 160, 320, 322
UCALL = NC * 4 * BPC         # 144: u_c[p,(k,g,b)]
CCOLS = C_UC + UCALL         # 466


def _build_kernel(tc: "tile.TileContext", out_ap, WUB, CONST, BPM):
    nc = tc.nc
    with (
        tc.tile_pool(name="wpool", bufs=4) as wpool,
        tc.tile_pool(name="tapool", bufs=4) as tapool,
        tc.tile_pool(name="cbpool", bufs=4) as cbpool,
        tc.tile_pool(name="persist", bufs=1) as persist,
        tc.tile_pool(name="psum_v", bufs=4, space="PSUM") as psum_v,
        tc.tile_pool(name="psum_s", bufs=1, space="PSUM") as psum_s,
        tc.tile_pool(name="psum_S2", bufs=1, space="PSUM") as psum_S2,
    ):
        const_t = persist.tile([128, CCOLS], BF16, tag="const")
        nc.sync.dma_start(const_t[:], CONST)
        bsel = const_t[:, C_BSEL:C_BSEL + 128]
        diag = const_t[:, C_DIAG:C_DIAG + 32]
        dmask = const_t[:, C_DMASK:C_DMASK + FD]
        bias2 = const_t[:, C_BIAS2:C_BIAS2 + 2]
        u_c_all = const_t[:, C_UC:C_UC + UCALL]

        # block-diagonal vote stationaries, built in per-batch pieces so the
        # first votes start right after the (tiny) const DMA:
        # ublk_all[p,(k,g,b,n')] = u_c[p,(k,g,b)] * diag[p,(b,n')]
        ublk_all = persist.tile([128, NC * 4 * BPC * 16], BF16, tag="ublk")
        d_vb = (
            diag.rearrange("p (b n) -> p b n", b=BPC, n=16)
            .unsqueeze(1)
            .unsqueeze(1)
            .broadcast_to([128, KB, 4, BPC, 16])
        )
        for kb in range(NB):
            ub_v = ublk_all[
                :, kb * KB * 128:(kb + 1) * KB * 128
            ].rearrange("p (k g b n) -> p k g b n", k=KB, g=4, b=BPC, n=16)
            u_cv = (
                u_c_all[:, kb * KB * 8:(kb + 1) * KB * 8]
                .rearrange("p (k g b) -> p k g b", k=KB, g=4, b=BPC)
                .unsqueeze(4)
                .broadcast_to([128, KB, 4, BPC, 16])
            )
            if kb % 2 == 0:
                nc.vector.tensor_tensor(ub_v, u_cv, d_vb, OP.mult)
            else:
                nc.gpsimd.tensor_tensor(ub_v, u_cv, d_vb, OP.mult)

        # preload the Exp ACT table while ACT is idle
        warm_t = persist.tile([1, 1], F32, tag="warm")
        nc.vector.memset(warm_t[:], 0.0)
        nc.scalar.activation(warm_t[:], warm_t[:], ACTF.Exp)

        u2bf_all = persist.tile([128, NC * FD], BF16, tag="u2bfall")
        s_bc = persist.tile([128, FD], BF16, tag="sbc")
        A_all = persist.tile([128, NC * D], F32, tag="aall")
        e_all = persist.tile([128, NC * NBD], F32, tag="eall")
        z_all = persist.tile([128, NC], F32, tag="zall")
        zr_all = persist.tile([128, NC], F32, tag="zrall")

        sbc_ps = psum_s.tile([128, 2 * FD], F32, tag="sbcps")
        S2_ps = psum_S2.tile([NBD, FD], F32, tag="S2ps")

        # ---- phase 1: batched W DMA; votes + s on PE; drain ----
        # sbc matmuls lag one batch behind the votes: emitted inline they
        # would wait on the drain inside the PE's in-order queue and stall
        # the following votes. Votes pack 2 chunks per PSUM bank so drains
        # and sbc run at 2-chunk granularity (half the ops and sem hops).
        def emit_sbc(kh):
            nc.tensor.matmul(
                sbc_ps[:],
                bsel,
                u2bf_all[:, 2 * kh * FD:(2 * kh + 2) * FD],
                start=(kh == 0),
                stop=(kh == NC // 2 - 1),
            )

        for kb in range(NB):
            w_t = wpool.tile([128, KB * WCOLS], BF16, tag="w")
            nc.sync.dma_start(w_t[:], WUB[kb])
            for kk in range(KB):
                k = kb * KB + kk
                if k % 2 == 0:
                    vote_ps = psum_v.tile([128, 2 * FD], F32, tag="vote")
                half = vote_ps[:, (k % 2) * FD:(k % 2 + 1) * FD]
                for g in range(4):
                    nc.tensor.matmul(
                        half[32 * g:32 * (g + 1), :],
                        ublk_all[:, (k * 4 + g) * 32:(k * 4 + g + 1) * 32],
                        w_t[:, (kk * 4 + g) * FD:(kk * 4 + g + 1) * FD],
                        start=True,
                        stop=True,
                        tile_position=(0, 32 * g),
                        skip_group_check=True,
                    )
                if k % 2 == 1:
                    u2_sl = u2bf_all[:, (k - 1) * FD:(k + 1) * FD]
                    if k % 4 == 1:
                        nc.scalar.copy(u2_sl, vote_ps[:])
                    else:
                        nc.vector.tensor_copy(u2_sl, vote_ps[:])
            # emit lagged sbc for chunk-pairs fully drained two batches ago
            done = max(0, ((kb - 1) * KB) // 2)
            prev = max(0, ((kb - 2) * KB) // 2)
            for kh in range(prev, done):
                emit_sbc(kh)
        for kh in range(max(0, (NB - 2) * KB // 2), NC // 2):
            emit_sbc(kh)

        # phase-2 constants: queued after the W batches so they don't delay
        # the vote-critical stream
        bpm_all = persist.tile([128, NC * NBD], BF16, tag="bpm")
        nc.sync.dma_start(bpm_all[:], BPM)

        # fold the two accumulated chunk-half columns: s = half0 + half1
        # (walrus allows at most one PSUM input per op: stage half1 in SBUF)
        s_h1 = persist.tile([128, FD], F32, tag="sh1")
        nc.scalar.copy(s_h1[:], sbc_ps[:, FD:])
        nc.vector.tensor_tensor(s_bc[:], sbc_ps[:, :FD], s_h1[:], OP.add)

        # ---- phase 2 (stage loops so each engine streams independent work
        # instead of stalling its in-order queue on cross-engine chains) ----
        # stage A: TA = U2*s_bc and A = sum_j TA, two chunks per op
        for kp in range(NC // 2):
            u2_sl = u2bf_all[:, 2 * kp * FD:(2 * kp + 2) * FD]
            ta = tapool.tile([128, 2 * FD], BF16, tag="ta")
            s2v = s_bc[:].unsqueeze(1).broadcast_to([128, 2, FD])
            u2v = u2_sl.rearrange("p (kk f) -> p kk f", kk=2, f=FD)
            ta_v = ta[:].rearrange("p (kk f) -> p kk f", kk=2, f=FD)
            # Pool is ~3x slower per elem: give it 4 of 9 pairs, DVE (which
            # also owns the reduces) the rest
            if kp % 2 == 1:
                nc.gpsimd.tensor_tensor(ta_v, u2v, s2v, OP.mult)
            else:
                nc.vector.tensor_tensor(ta_v, u2v, s2v, OP.mult)
            a_sl = A_all[:, 2 * kp * D:(2 * kp + 2) * D]
            ta_dj = ta[:].rearrange("p (kd j) -> p kd j", kd=2 * D, j=DD)
            nc.vector.tensor_reduce(a_sl, ta_dj, AX.X, OP.add)
        # stage B: masked exp: e[p,(kk,b',d)] = exp(A*s + bias2[b'])
        # (bias -30000 where b(p) != b' -> dead half exactly 0); 6 chunks/op
        EK = 6
        for kp in range(NC // EK):
            k = EK * kp
            a_sl6 = A_all[:, k * D:(k + EK) * D].rearrange(
                "p (kk d) -> p kk d", kk=EK, d=D
            )
            e_sl6 = e_all[:, k * NBD:(k + EK) * NBD].rearrange(
                "p (kk b d) -> p kk b d", kk=EK, b=BPC, d=D
            )
            for b2 in range(BPC):
                nc.scalar.activation(
                    e_sl6[:, :, b2, :],
                    a_sl6,
                    ACTF.Exp,
                    bias=bias2[:, b2:b2 + 1],
                    scale=INV_SQRT_DP,
                )
        # stage C: z = sum_(b',d) e at exp-group granularity; zr = 1/z
        for kp in range(NC // EK):
            k = EK * kp
            e_v = e_all[:, k * NBD:(k + EK) * NBD].rearrange(
                "p (kk m) -> p kk m", kk=EK, m=NBD
            )
            nc.vector.tensor_reduce(z_all[:, k:k + EK], e_v, AX.X, OP.add)
            nc.vector.reciprocal(zr_all[:, k:k + EK], z_all[:, k:k + EK])
        # stage D: all cbD = e*zr + BpM (bf16) first, then the S2 matmuls
        # stream on a warm PE without per-chunk cross-engine stalls
        cbD_all = persist.tile([128, NC * NBD], BF16, tag="cbDall")
        for k in range(NC):
            nc.vector.scalar_tensor_tensor(
                cbD_all[:, k * NBD:(k + 1) * NBD],
                e_all[:, k * NBD:(k + 1) * NBD],
                zr_all[:, k:k + 1],
                bpm_all[:, k * NBD:(k + 1) * NBD],
                OP.mult,
                OP.add,
            )
        for k in range(NC):
            nc.tensor.matmul(
                S2_ps[:],
                cbD_all[:, k * NBD:(k + 1) * NBD],
                u2bf_all[:, k * FD:(k + 1) * FD],
                start=(k == 0),
                stop=(k == NC - 1),
            )

        # ---- phase 3: extract diagonal d(row)==d' and squash ----
        sm_t = persist.tile([NBD, FD], F32, tag="sm")
        nc.vector.tensor_tensor(sm_t[:], S2_ps[:], dmask[:NBD, :], OP.mult)
        s_diag = persist.tile([NBD, DD], F32, tag="sdiag")
        nc.vector.tensor_reduce(
            s_diag[:],
            sm_t[:].rearrange("p (g j) -> p j g", g=D, j=DD),
            AX.X,
            OP.add,
        )

        ss_t = persist.tile([NBD, DD], F32, tag="ss")
        nrm2 = persist.tile([NBD, 1], F32, tag="nrm2")
        nc.vector.tensor_tensor(ss_t[:], s_diag[:], s_diag[:], OP.mult)
        nc.vector.tensor_reduce(nrm2[:], ss_t[:], AX.X, OP.add)
        # norm via one Halley iteration from a bit-hack seed, all on DVE
        # (keeps the Exp ACT table resident - no sqrt table load)
        nrm = persist.tile([NBD, 1], F32, tag="nrm")
        seed_i = persist.tile([NBD, 1], I32, tag="seedi")
        nc.vector.tensor_scalar(
            seed_i[:], nrm2[:].bitcast(I32), 1, None, OP.logical_shift_right
        )
        nc.vector.tensor_scalar(seed_i[:], seed_i[:], 0x1FBD1DF5, None, OP.add)
        seed_f = seed_i[:].bitcast(F32)
        y2 = persist.tile([NBD, 1], F32, tag="y2")
        nc.vector.tensor_tensor(y2[:], seed_f, seed_f, OP.mult)
        hnum = persist.tile([NBD, 1], F32, tag="hnum")
        nc.vector.scalar_tensor_tensor(hnum[:], nrm2[:], 3.0, y2[:], OP.mult, OP.add)
        hden = persist.tile([NBD, 1], F32, tag="hden")
        nc.vector.scalar_tensor_tensor(hden[:], y2[:], 3.0, nrm2[:], OP.mult, OP.add)
        nwr = persist.tile([NBD, 1], F32, tag="nwr")
        nc.vector.reciprocal(nwr[:], hden[:])
        nwt = persist.tile([NBD, 1], F32, tag="nwt")
        nc.vector.tensor_tensor(nwt[:], hnum[:], nwr[:], OP.mult)
        nc.vector.tensor_tensor(nrm[:], seed_f, nwt[:], OP.mult)
        # coef = 1 - 1/(e^r + eps) ~= 1 - e^-r
        en = persist.tile([NBD, 1], F32, tag="en")
        nc.scalar.activation(en[:], nrm[:], ACTF.Exp, scale=-1.0)
        coef = persist.tile([NBD, 1], F32, tag="coef")
        nc.vector.tensor_scalar(coef[:], en[:], -1.0, 1.0, OP.mult, OP.add)
        r2 = persist.tile([NBD, 1], F32, tag="r2")
        nc.vector.reciprocal(r2[:], nrm[:])
        fac = persist.tile([NBD, 1], F32, tag="fac")
        nc.vector.tensor_tensor(fac[:], coef[:], r2[:], OP.mult)

        res_t = persist.tile([NBD, DD], F32, tag="res")
        nc.vector.tensor_scalar(res_t[:], s_diag[:], fac[:], None, OP.mult)

        nc.sync.dma_start(out_ap.rearrange("b d j -> (b d) j"), res_t[:])


_CACHE: dict = {}


def _get_nc():
    if "nc" not in _CACHE:
        nc = bacc.Bacc(
            "TRN2", target_bir_lowering=False, debug=False, num_devices=NCORES
        )
        WUB = nc.dram_tensor(
            "wub_arr", [NB, 128, KB * WCOLS], BF16, kind="ExternalInput"
        ).ap()
        CONST = nc.dram_tensor(
            "const_arr", [128, CCOLS], BF16, kind="ExternalInput"
        ).ap()
        BPM = nc.dram_tensor(
            "bpm_arr", [128, NC * NBD], BF16, kind="ExternalInput"
        ).ap()
        out = nc.dram_tensor("out", [BPC, D, DD], F32, kind="ExternalOutput").ap()
        with tile.TileContext(nc) as tc:
            _build_kernel(tc, out, WUB, CONST, BPM)
        nc.compile()
        _CACHE["nc"] = nc
    return _CACHE["nc"]


def _np_bf16(x):
    import ml_dtypes

    return np.asarray(x, dtype=np.float32).astype(ml_dtypes.bfloat16)


def _const_arr(pc):
    p = np.arange(128)
    bp = (p // 16) % 2                       # b(p) within chunk partitions
    const = np.zeros((128, CCOLS), dtype=np.float32)
    # bsel[p,p'] = delta(b(p)==b(p'))
    const[:, C_BSEL:C_BSEL + 128] = (bp[:, None] == bp[None, :])
    # diag[p,(b,n')] = delta(p//8 == n')
    nn = np.arange(16)
    dg = (p[:, None] // 8 == nn[None, :]).astype(np.float32)  # [128,16]
    const[:, C_DIAG:C_DIAG + 32] = np.concatenate([dg, dg], axis=1)
    # dmask[r,(d',j)] = delta(r%10 == d') for r<20
    r = np.arange(NBD)
    dmask = (r[:, None] % D == np.arange(D)[None, :]).astype(np.float32)
    const[:NBD, C_DMASK:C_DMASK + FD] = np.repeat(dmask, DD, axis=1)
    # bias2[p,b'] = 0 if b(p)==b' else -30000 (kills the dead exp half)
    const[:, C_BIAS2:C_BIAS2 + 2] = np.where(
        bp[:, None] == np.arange(BPC)[None, :], 0.0, -30000.0
    )
    # u_c_all[p=(ns,i), (k,g,b)] = u[b, 16*(4k+g)+ns, i]
    uv = pc.reshape(BPC, NC, 4, 16, DP)                      # [b,k,g,ns,i]
    const[:, C_UC:] = uv.transpose(3, 4, 1, 2, 0).reshape(128, UCALL)
    return _np_bf16(const)


def _arrange(primary_caps, W, B_prior, core):
    """Host-side pre-arrangement into per-batch contiguous bf16 layouts.

    chunk k partitions p=(ns,i): ns=p//8, i=p%8.
    wub[kb, p, (kk,g,d,j)] = W[d, n(3kb+kk, g, ns), j, i]
    """
    W = np.asarray(W, dtype=np.float32)
    Bp = np.asarray(B_prior, dtype=np.float32)[:, 0, :]      # [D, N]
    pc = np.asarray(primary_caps, dtype=np.float32)[
        core * BPC:(core + 1) * BPC
    ]                                                        # [2, N, DP]

    # W part: [D,N,DD,DP] -> [k, (ns,i), (g, d, j)] -> batches of KB chunks
    Wv = W.transpose(1, 3, 0, 2).reshape(NC, 4, 16, DP, D, DD)
    wub = (
        Wv.transpose(0, 2, 3, 1, 4, 5)
        .reshape(NC, 128, WCOLS)
        .reshape(NB, KB, 128, WCOLS)
        .transpose(0, 2, 1, 3)
        .reshape(NB, 128, KB * WCOLS)
    )
    # BpM: [p=(g2,b2,n'), (k, b', d)] = Bp[d, 16*(4k+g2)+n'] * delta(b2==b')
    bpv = Bp.T.reshape(NC, 4, 16, D)                         # [k, g2, n', d]
    bpp = bpv.transpose(0, 1, 3, 2)                          # [k, g2, d, n']
    bpp = np.repeat(bpp[:, :, None, :, :], BPC, axis=2)      # [k,g2,b2,d,n']
    bpp = bpp.transpose(0, 1, 2, 4, 3).reshape(NC, 128, D)   # [k, p, d]
    bsel2 = np.zeros((128, BPC), dtype=np.float32)
    bsel2[np.arange(128), ((np.arange(128) // 16) % 2)] = 1.0
    bpm = (
        (bsel2[None, :, :, None] * bpp[:, :, None, :])       # [k, p, b', d]
        .reshape(NC, 128, NBD)
        .transpose(1, 0, 2)
        .reshape(128, NC * NBD)
    )
    return {
        "wub_arr": _np_bf16(wub),
        "const_arr": _const_arr(pc),
        "bpm_arr": _np_bf16(bpm),
    }


def _run(primary_caps, W, B_prior, trace=False, **kw):
    nc = _get_nc()
    in_maps = [
        _arrange(primary_caps, W, B_prior, c) for c in range(NCORES)
    ]
    res = run_bass_kernel_spmd(nc, in_maps, list(range(NCORES)), trace=trace, **kw)
    out = np.concatenate([res.results[c]["out"] for c in range(NCORES)], axis=0)
    return out.astype(np.float32), res


def kernel(primary_caps, W, B_prior):
    out, _ = _run(primary_caps, W, B_prior, trace=False)
    return out


# revision 46
# speedup vs baseline: 1.0012x; 1.0012x over previous
"""DigitCaps kernel for 8 Trainium2 NeuronCores — PE-voting redesign.

Math (per batch b):
    U_hat[b,d,n,j] = sum_i W[d,n,j,i] * u[b,n,i]
    s[b,d,j]       = sum_n U_hat[b,d,n,j]
    A_sum[b,d,m]   = s[b,d,:] . U_hat[b,d,m,:] / sqrt(dp)
    C              = softmax_d(A_sum)
    S[b,d,j]       = sum_m (B_prior[d,m] + C[b,d,m]) * U_hat[b,d,m,j]
    out            = squash(S)

Sharding: data-parallel over batch, 2 batches per core; W/B_prior replicated
in bf16 (host-cast; rel err ~4e-3 << 2e-2 tolerance, halves the HBM read).

Votes run on the PE via a block-diagonal stationary operand: partitions hold
(n_sub=16, i=8) of a 16-n group; lhsT = u_blk[128, (b=2, n')=32] with
u_blk[(ns,i),(b,n')] = u[b,n(ns),i] * delta(ns==n'), rhs = W[128, (d,j)=160].
Out = U_hat[(b,n'), (d,j)] written to a 32-partition slice of PSUM; 4 groups
pack one bank to 128 partitions (a 64-n "chunk"; 18 chunks per core).

s is accumulated on the PE as bsel.T @ U2 (bsel[p,p'] = delta(b(p)==b(p')))
which lands s directly in row-broadcast layout [p=(b,m), (d,j)].

Phase 2 per chunk in layout [p=(g_sub,b,n'), (d,j)]: softmax over d is a
free-dim strided reduce (no cross-partition work); cbD = (exp*zr + Bp*bmask)
in one fused DVE op (Bp premasked on host); S via accumulating bf16 matmul
with lhsT = cbD[128, (b',d)=20]; diagonal d==d' extracted with a host mask.
"""

import math
import numpy as np

import concourse.bacc as bacc
import concourse.bass as bass
import concourse.tile as tile
from concourse import mybir
from concourse.bass_utils import run_bass_kernel_spmd

F32 = mybir.dt.float32
BF16 = mybir.dt.bfloat16
I32 = mybir.dt.int32
AX = mybir.AxisListType
OP = mybir.AluOpType
ACTF = mybir.ActivationFunctionType

B, N, DP = 16, 1152, 8
D, DD = 10, 16
NCORES = 8
BPC = B // NCORES            # 2 batches per core
FD = D * DD                  # 160 = (d,j)
NG = N // 16                 # 72 groups of 16 n's
NC = NG // 4                 # 18 chunks of 4 groups (64 n's)
NBD = BPC * D                # 20 = (b,d)
EPS = 1e-7
INV_SQRT_DP = 1.0 / math.sqrt(DP)

# W batches: KB chunks per DMA (fewer DMAs amortize the ~625ns serial HWDGE
# descriptor-gen per DMACopy; chunk-granular DMAs spent 13us in HWDGE)
KB = 3                       # chunks per W DMA batch
NB = NC // KB                # 6 batches
WCOLS = 4 * FD               # 640 W cols per chunk

# const tile free layout (bf16 cols):
# bsel 128 | diag 32 | dmask 160 | bias2 2 | u_c_all 144
C_BSEL, C_DIAG, C_DMASK, C_BIAS2, C_UC = 0, 128, 160, 320, 322
UCALL = NC * 4 * BPC         # 144: u_c[p,(k,g,b)]
CCOLS = C_UC + UCALL         # 466


def _build_kernel(tc: "tile.TileContext", out_ap, WUB, CONST, BPM):
    nc = tc.nc
    with (
        tc.tile_pool(name="wpool", bufs=4) as wpool,
        tc.tile_pool(name="tapool", bufs=4) as tapool,
        tc.tile_pool(name="cbpool", bufs=4) as cbpool,
        tc.tile_pool(name="persist", bufs=1) as persist,
        tc.tile_pool(name="psum_v", bufs=4, space="PSUM") as psum_v,
        tc.tile_pool(name="psum_s", bufs=1, space="PSUM") as psum_s,
        tc.tile_pool(name="psum_S2", bufs=1, space="PSUM") as psum_S2,
    ):
        const_t = persist.tile([128, CCOLS], BF16, tag="const")
        nc.sync.dma_start(const_t[:], CONST)
        bsel = const_t[:, C_BSEL:C_BSEL + 128]
        diag = const_t[:, C_DIAG:C_DIAG + 32]
        dmask = const_t[:, C_DMASK:C_DMASK + FD]
        bias2 = const_t[:, C_BIAS2:C_BIAS2 + 2]
        u_c_all = const_t[:, C_UC:C_UC + UCALL]

        # block-diagonal vote stationaries, built in per-batch pieces so the
        # first votes start right after the (tiny) const DMA:
        # ublk_all[p,(k,g,b,n')] = u_c[p,(k,g,b)] * diag[p,(b,n')]
        ublk_all = persist.tile([128, NC * 4 * BPC * 16], BF16, tag="ublk")
        d_vb = (
            diag.rearrange("p (b n) -> p b n", b=BPC, n=16)
            .unsqueeze(1)
            .unsqueeze(1)
            .broadcast_to([128, KB, 4, BPC, 16])
        )
        for kb in range(NB):
            ub_v = ublk_all[
                :, kb * KB * 128:(kb + 1) * KB * 128
            ].rearrange("p (k g b n) -> p k g b n", k=KB, g=4, b=BPC, n=16)
            u_cv = (
                u_c_all[:, kb * KB * 8:(kb + 1) * KB * 8]
                .rearrange("p (k g b) -> p k g b", k=KB, g=4, b=BPC)
                .unsqueeze(4)
                .broadcast_to([128, KB, 4, BPC, 16])
            )
            if kb % 2 == 0:
                nc.vector.tensor_tensor(ub_v, u_cv, d_vb, OP.mult)
            else:
                nc.gpsimd.tensor_tensor(ub_v, u_cv, d_vb, OP.mult)

        # preload the Exp ACT table while ACT is idle
        warm_t = persist.tile([1, 1], F32, tag="warm")
        nc.vector.memset(warm_t[:], 0.0)
        nc.scalar.activation(warm_t[:], warm_t[:], ACTF.Exp)

        u2bf_all = persist.tile([128, NC * FD], BF16, tag="u2bfall")
        s_bc = persist.tile([128, FD], BF16, tag="sbc")
        A_all = persist.tile([128, NC * D], F32, tag="aall")
        e_all = persist.tile([128, NC * NBD], F32, tag="eall")
        z_all = persist.tile([128, NC], F32, tag="zall")
        zr_all = persist.tile([128, NC], F32, tag="zrall")

        sbc_ps = psum_s.tile([128, FD], F32, tag="sbcps")
        S2_ps = psum_S2.tile([NBD, FD], F32, tag="S2ps")

        # ---- phase 1: batched W DMA; votes + s on PE; drain ----
        # sbc matmuls lag one batch behind the votes: emitted inline they
        # would wait on the drain inside the PE's in-order queue and stall
        # the following votes. Votes pack 2 chunks per PSUM bank so drains
        # and sbc run at 2-chunk granularity (half the ops and sem hops).
        def emit_sbc(kh):
            for k in (2 * kh, 2 * kh + 1):
                nc.tensor.matmul(
                    sbc_ps[:],
                    bsel,
                    u2bf_all[:, k * FD:(k + 1) * FD],
                    start=(k == 0),
                    stop=(k == NC - 1),
                )

        for kb in range(NB):
            w_t = wpool.tile([128, KB * WCOLS], BF16, tag="w")
            nc.sync.dma_start(w_t[:], WUB[kb])
            for kk in range(KB):
                k = kb * KB + kk
                if k % 2 == 0:
                    vote_ps = psum_v.tile([128, 2 * FD], F32, tag="vote")
                half = vote_ps[:, (k % 2) * FD:(k % 2 + 1) * FD]
                for g in range(4):
                    nc.tensor.matmul(
                        half[32 * g:32 * (g + 1), :],
                        ublk_all[:, (k * 4 + g) * 32:(k * 4 + g + 1) * 32],
                        w_t[:, (kk * 4 + g) * FD:(kk * 4 + g + 1) * FD],
                        start=True,
                        stop=True,
                        tile_position=(0, 32 * g),
                        skip_group_check=True,
                    )
                if k % 2 == 1:
                    u2_sl = u2bf_all[:, (k - 1) * FD:(k + 1) * FD]
                    if k % 4 == 1:
                        nc.scalar.copy(u2_sl, vote_ps[:])
                    else:
                        nc.vector.tensor_copy(u2_sl, vote_ps[:])
            # emit lagged sbc for chunk-pairs fully drained two batches ago
            done = max(0, ((kb - 1) * KB) // 2)
            prev = max(0, ((kb - 2) * KB) // 2)
            for kh in range(prev, done):
                emit_sbc(kh)
        for kh in range(max(0, (NB - 2) * KB // 2), NC // 2):
            emit_sbc(kh)

        # phase-2 constants: queued after the W batches so they don't delay
        # the vote-critical stream
        bpm_all = persist.tile([128, NC * NBD], BF16, tag="bpm")
        nc.sync.dma_start(bpm_all[:], BPM)

        nc.scalar.copy(s_bc[:], sbc_ps[:])

        # ---- phase 2 (stage loops so each engine streams independent work
        # instead of stalling its in-order queue on cross-engine chains) ----
        # stage A: TA = U2*s_bc and A = sum_j TA, two chunks per op
        for kp in range(NC // 2):
            u2_sl = u2bf_all[:, 2 * kp * FD:(2 * kp + 2) * FD]
            ta = tapool.tile([128, 2 * FD], BF16, tag="ta")
            s2v = s_bc[:].unsqueeze(1).broadcast_to([128, 2, FD])
            u2v = u2_sl.rearrange("p (kk f) -> p kk f", kk=2, f=FD)
            ta_v = ta[:].rearrange("p (kk f) -> p kk f", kk=2, f=FD)
            # Pool is ~3x slower per elem: give it 4 of 9 pairs, DVE (which
            # also owns the reduces) the rest
            if kp % 2 == 1:
                nc.gpsimd.tensor_tensor(ta_v, u2v, s2v, OP.mult)
            else:
                nc.vector.tensor_tensor(ta_v, u2v, s2v, OP.mult)
            a_sl = A_all[:, 2 * kp * D:(2 * kp + 2) * D]
            ta_dj = ta[:].rearrange("p (kd j) -> p kd j", kd=2 * D, j=DD)
            nc.vector.tensor_reduce(a_sl, ta_dj, AX.X, OP.add)
        # stage B: masked exp: e[p,(kk,b',d)] = exp(A*s + bias2[b'])
        # (bias -30000 where b(p) != b' -> dead half exactly 0); 6 chunks/op
        EK = 6
        for kp in range(NC // EK):
            k = EK * kp
            a_sl6 = A_all[:, k * D:(k + EK) * D].rearrange(
                "p (kk d) -> p kk d", kk=EK, d=D
            )
            e_sl6 = e_all[:, k * NBD:(k + EK) * NBD].rearrange(
                "p (kk b d) -> p kk b d", kk=EK, b=BPC, d=D
            )
            for b2 in range(BPC):
                nc.scalar.activation(
                    e_sl6[:, :, b2, :],
                    a_sl6,
                    ACTF.Exp,
                    bias=bias2[:, b2:b2 + 1],
                    scale=INV_SQRT_DP,
                )
        # stage C: z = sum_(b',d) e at exp-group granularity; zr = 1/z
        for kp in range(NC // EK):
            k = EK * kp
            e_v = e_all[:, k * NBD:(k + EK) * NBD].rearrange(
                "p (kk m) -> p kk m", kk=EK, m=NBD
            )
            nc.vector.tensor_reduce(z_all[:, k:k + EK], e_v, AX.X, OP.add)
            nc.vector.reciprocal(zr_all[:, k:k + EK], z_all[:, k:k + EK])
        # stage D: all cbD = e*zr + BpM (bf16) first, then the S2 matmuls
        # stream on a warm PE without per-chunk cross-engine stalls
        cbD_all = persist.tile([128, NC * NBD], BF16, tag="cbDall")
        for k in range(NC):
            nc.vector.scalar_tensor_tensor(
                cbD_all[:, k * NBD:(k + 1) * NBD],
                e_all[:, k * NBD:(k + 1) * NBD],
                zr_all[:, k:k + 1],
                bpm_all[:, k * NBD:(k + 1) * NBD],
                OP.mult,
                OP.add,
            )
        for k in range(NC):
            nc.tensor.matmul(
                S2_ps[:],
                cbD_all[:, k * NBD:(k + 1) * NBD],
                u2bf_all[:, k * FD:(k + 1) * FD],
                start=(k == 0),
                stop=(k == NC - 1),
            )

        # ---- phase 3: extract diagonal d(row)==d' and squash ----
        sm_t = persist.tile([NBD, FD], F32, tag="sm")
        nc.vector.tensor_tensor(sm_t[:], S2_ps[:], dmask[:NBD, :], OP.mult)
        s_diag = persist.tile([NBD, DD], F32, tag="sdiag")
        nc.vector.tensor_reduce(
            s_diag[:],
            sm_t[:].rearrange("p (g j) -> p j g", g=D, j=DD),
            AX.X,
            OP.add,
        )

        ss_t = persist.tile([NBD, DD], F32, tag="ss")
        nrm2 = persist.tile([NBD, 1], F32, tag="nrm2")
        nc.vector.tensor_tensor(ss_t[:], s_diag[:], s_diag[:], OP.mult)
        nc.vector.tensor_reduce(nrm2[:], ss_t[:], AX.X, OP.add)
        # norm via one Halley iteration from a bit-hack seed, all on DVE
        # (keeps the Exp ACT table resident - no sqrt table load)
        nrm = persist.tile([NBD, 1], F32, tag="nrm")
        seed_i = persist.tile([NBD, 1], I32, tag="seedi")
        nc.vector.tensor_scalar(
            seed_i[:], nrm2[:].bitcast(I32), 1, None, OP.logical_shift_right
        )
        nc.vector.tensor_scalar(seed_i[:], seed_i[:], 0x1FBD1DF5, None, OP.add)
        seed_f = seed_i[:].bitcast(F32)
        y2 = persist.tile([NBD, 1], F32, tag="y2")
        nc.vector.tensor_tensor(y2[:], seed_f, seed_f, OP.mult)
        hnum = persist.tile([NBD, 1], F32, tag="hnum")
        nc.vector.scalar_tensor_tensor(hnum[:], nrm2[:], 3.0, y2[:], OP.mult, OP.add)
        hden = persist.tile([NBD, 1], F32, tag="hden")
        nc.vector.scalar_tensor_tensor(hden[:], y2[:], 3.0, nrm2[:], OP.mult, OP.add)
        nwr = persist.tile([NBD, 1], F32, tag="nwr")
        nc.vector.reciprocal(nwr[:], hden[:])
        nwt = persist.tile([NBD, 1], F32, tag="nwt")
        nc.vector.tensor_tensor(nwt[:], hnum[:], nwr[:], OP.mult)
        nc.vector.tensor_tensor(nrm[:], seed_f, nwt[:], OP.mult)
        # coef = 1 - 1/(e^r + eps) ~= 1 - e^-r
        en = persist.tile([NBD, 1], F32, tag="en")
        nc.scalar.activation(en[:], nrm[:], ACTF.Exp, scale=-1.0)
        coef = persist.tile([NBD, 1], F32, tag="coef")
        nc.vector.tensor_scalar(coef[:], en[:], -1.0, 1.0, OP.mult, OP.add)
        r2 = persist.tile([NBD, 1], F32, tag="r2")
        nc.vector.reciprocal(r2[:], nrm[:])
        fac = persist.tile([NBD, 1], F32, tag="fac")
        nc.vector.tensor_tensor(fac[:], coef[:], r2[:], OP.mult)

        res_t = persist.tile([NBD, DD], F32, tag="res")
        nc.vector.tensor_scalar(res_t[:], s_diag[:], fac[:], None, OP.mult)

        nc.sync.dma_start(out_ap.rearrange("b d j -> (b d) j"), res_t[:])


_CACHE: dict = {}


def _get_nc():
    if "nc" not in _CACHE:
        nc = bacc.Bacc(
            "TRN2", target_bir_lowering=False, debug=False, num_devices=NCORES
        )
        WUB = nc.dram_tensor(
            "wub_arr", [NB, 128, KB * WCOLS], BF16, kind="ExternalInput"
        ).ap()
        CONST = nc.dram_tensor(
            "const_arr", [128, CCOLS], BF16, kind="ExternalInput"
        ).ap()
        BPM = nc.dram_tensor(
            "bpm_arr", [128, NC * NBD], BF16, kind="ExternalInput"
        ).ap()
        out = nc.dram_tensor("out", [BPC, D, DD], F32, kind="ExternalOutput").ap()
        with tile.TileContext(nc) as tc:
            _build_kernel(tc, out, WUB, CONST, BPM)
        nc.compile()
        _CACHE["nc"] = nc
    return _CACHE["nc"]


def _np_bf16(x):
    import ml_dtypes

    return np.asarray(x, dtype=np.float32).astype(ml_dtypes.bfloat16)


def _const_arr(pc):
    p = np.arange(128)
    bp = (p // 16) % 2                       # b(p) within chunk partitions
    const = np.zeros((128, CCOLS), dtype=np.float32)
    # bsel[p,p'] = delta(b(p)==b(p'))
    const[:, C_BSEL:C_BSEL + 128] = (bp[:, None] == bp[None, :])
    # diag[p,(b,n')] = delta(p//8 == n')
    nn = np.arange(16)
    dg = (p[:, None] // 8 == nn[None, :]).astype(np.float32)  # [128,16]
    const[:, C_DIAG:C_DIAG + 32] = np.concatenate([dg, dg], axis=1)
    # dmask[r,(d',j)] = delta(r%10 == d') for r<20
    r = np.arange(NBD)
    dmask = (r[:, None] % D == np.arange(D)[None, :]).astype(np.float32)
    const[:NBD, C_DMASK:C_DMASK + FD] = np.repeat(dmask, DD, axis=1)
    # bias2[p,b'] = 0 if b(p)==b' else -30000 (kills the dead exp half)
    const[:, C_BIAS2:C_BIAS2 + 2] = np.where(
        bp[:, None] == np.arange(BPC)[None, :], 0.0, -30000.0
    )
    # u_c_all[p=(ns,i), (k,g,b)] = u[b, 16*(4k+g)+ns, i]
    uv = pc.reshape(BPC, NC, 4, 16, DP)                      # [b,k,g,ns,i]
    const[:, C_UC:] = uv.transpose(3, 4, 1, 2, 0).reshape(128, UCALL)
    return _np_bf16(const)


def _arrange(primary_caps, W, B_prior, core):
    """Host-side pre-arrangement into per-batch contiguous bf16 layouts.

    chunk k partitions p=(ns,i): ns=p//8, i=p%8.
    wub[kb, p, (kk,g,d,j)] = W[d, n(3kb+kk, g, ns), j, i]
    """
    W = np.asarray(W, dtype=np.float32)
    Bp = np.asarray(B_prior, dtype=np.float32)[:, 0, :]      # [D, N]
    pc = np.asarray(primary_caps, dtype=np.float32)[
        core * BPC:(core + 1) * BPC
    ]                                                        # [2, N, DP]

    # W part: [D,N,DD,DP] -> [k, (ns,i), (g, d, j)] -> batches of KB chunks
    Wv = W.transpose(1, 3, 0, 2).reshape(NC, 4, 16, DP, D, DD)
    wub = (
        Wv.transpose(0, 2, 3, 1, 4, 5)
        .reshape(NC, 128, WCOLS)
        .reshape(NB, KB, 128, WCOLS)
        .transpose(0, 2, 1, 3)
        .reshape(NB, 128, KB * WCOLS)
    )
    # BpM: [p=(g2,b2,n'), (k, b', d)] = Bp[d, 16*(4k+g2)+n'] * delta(b2==b')
    bpv = Bp.T.reshape(NC, 4, 16, D)                         # [k, g2, n', d]
    bpp = bpv.transpose(0, 1, 3, 2)                          # [k, g2, d, n']
    bpp = np.repeat(bpp[:, :, None, :, :], BPC, axis=2)      # [k,g2,b2,d,n']
    bpp = bpp.transpose(0, 1, 2, 4, 3).reshape(NC, 128, D)   # [k, p, d]
    bsel2 = np.zeros((128, BPC), dtype=np.float32)
    bsel2[np.arange(128), ((np.arange(128) // 16) % 2)] = 1.0
    bpm = (
        (bsel2[None, :, :, None] * bpp[:, :, None, :])       # [k, p, b', d]
        .reshape(NC, 128, NBD)
        .transpose(1, 0, 2)
        .reshape(128, NC * NBD)
    )
    return {
        "wub_arr": _np_bf16(wub),
        "const_arr": _const_arr(pc),
        "bpm_arr": _np_bf16(bpm),
    }


def _run(primary_caps, W, B_prior, trace=False, **kw):
    nc = _get_nc()
    in_maps = [
        _arrange(primary_caps, W, B_prior, c) for c in range(NCORES)
    ]
    res = run_bass_kernel_spmd(nc, in_maps, list(range(NCORES)), trace=trace, **kw)
    out = np.concatenate([res.results[c]["out"] for c in range(NCORES)], axis=0)
    return out.astype(np.float32), res


def kernel(primary_caps, W, B_prior):
    out, _ = _run(primary_caps, W, B_prior, trace=False)
    return out
